# revision 12
# baseline (speedup 1.0000x reference)
"""Trainium2 Bass kernel for nn_CrossAttn (two-branch full cross attention).

Problem (per branch, per batch):
    q = x_q.reshape(N, C); k = x_k.reshape(N, C)          # N=4096, C=256
    E = q @ k.T                                           # [N, N]
    A = softmax(-E, axis=-1)
    out = gamma * (A @ q) + q                             # values == q

Sharding: 8 independent work units = 2 branches x 4 batches -> one per
NeuronCore (pure SPMD, no collectives).

Per-core dataflow:
  - Load q, k naturally; build V' = [q(bf16) | ones-column] per 128-row chunk.
  - PE-transpose q, k into Q^T, K^T ([c on partitions, n free], fp32).
  - Scores are computed TRANSPOSED: E_T[m, n] = sum_c K[m,c] Q[n,c]
    (lhsT = K^T chunk, rhs = Q^T superblock, fp32r full-rate).
  - A_T = exp(-E_T - SHIFT) on ScalarE (softmax is shift-invariant, so a
    constant shift replaces the row-max pass; -E ~ N(0,256) keeps
    exp(-E-100) far away from both fp32 overflow and total underflow).
  - out' = A_T.T @ V' accumulated over key chunks in PSUM; the ones
    column makes out'[:, C] the softmax denominator for free.
  - out = gamma * out'[:, :C] / out'[:, C] + q  (VectorE epilogue).
"""

from contextlib import ExitStack

import numpy as np

import concourse.bass as bass
import concourse.bacc as bacc
import concourse.mybir as mybir
import concourse.tile as tile
from concourse.bass_utils import run_bass_kernel_spmd
from concourse.masks import make_identity

F32 = mybir.dt.float32
F32R = mybir.dt.float32r
BF16 = mybir.dt.bfloat16

B, H, W, C = 4, 64, 64, 256
N = H * W  # 4096
SHIFT = -100.0  # constant softmax shift: A = exp(-E + SHIFT)


def emit_cross_attn(ctx, tc, q, k, g, o, n, c, score_dtype=F32R):
    """Emit one core's cross-attention program.

    q, k: DRAM [n, c] fp32 (q is queries+values+residual, k is keys)
    g:    DRAM [1, 1] fp32 (gamma)
    o:    DRAM [n, c] fp32
    """
    nc = tc.nc
    P = 128
    n_blk = n // P          # 128-row chunks of q/k
    n_cch = c // P          # 128-col chunks of the feature dim
    SB = min(512, n)        # query superblock width
    n_sb = n // SB
    nb_per_sb = SB // P
    dma_chunks = 4 if n_blk % 4 == 0 else 1
    blk_per_dma = n_blk // dma_chunks

    persist = ctx.enter_context(tc.tile_pool(name="persist", bufs=1))
    small = ctx.enter_context(tc.tile_pool(name="small", bufs=8))
    atp = ctx.enter_context(tc.tile_pool(name="atp", bufs=3))
    opool = ctx.enter_context(tc.tile_pool(name="opool", bufs=4))

    # --- persistent SBUF tensors ---
    ident = persist.tile([P, P], F32, tag="ident")
    make_identity(nc, ident[:, :])
    shift_t = persist.tile([P, 1], F32, tag="shift")
    nc.vector.memset(shift_t[:, :], SHIFT)
    gt = persist.tile([P, 1], F32, tag="gamma")
    g_ap = g[:]
    nc.default_dma_engine.dma_start(
        out=gt[:, :],
        in_=bass.AP(tensor=g_ap.tensor, offset=0, ap=[[0, P], [1, 1]]),
    )

    qnat = persist.tile([P, n_blk, c], F32, tag="qnat")   # q natural [p, blk, c]
    knat = persist.tile([P, n_blk, c], F32, tag="knat")
    qt = persist.tile([P, n_cch, n], score_dtype, tag="qt")  # Q^T [c, cch, n]
    kt = persist.tile([P, n_cch, n], score_dtype, tag="kt")
    vt = persist.tile([P, n_blk, c + 1], BF16, tag="vt")  # V' [m-part, blk, c+1]

    # --- stage A: load + transpose + build V' ---
    q3 = q.rearrange("(i p) c -> p i c", p=P)
    k3 = k.rearrange("(i p) c -> p i c", p=P)
    for j in range(dma_chunks):
        sl = slice(j * blk_per_dma, (j + 1) * blk_per_dma)
        nc.default_dma_engine.dma_start(out=knat[:, sl, :], in_=k3[:, sl, :])
    for j in range(dma_chunks):
        sl = slice(j * blk_per_dma, (j + 1) * blk_per_dma)
        nc.default_dma_engine.dma_start(out=qnat[:, sl, :], in_=q3[:, sl, :])

    with tc.tile_pool(name="tpsum", bufs=4, space="PSUM") as tpsum:
        for i in range(n_blk):  # K first: all of K^T gates superblock 0
            for cc in range(n_cch):
                tp = tpsum.tile([P, P], F32, tag="tp")
                nc.tensor.transpose(tp[:, :], knat[:, i, cc * P:(cc + 1) * P],
                                    ident[:, :])
                if (i * n_cch + cc) % 2 == 0:
                    nc.vector.tensor_copy(kt[:, cc, i * P:(i + 1) * P], tp[:, :])
                else:
                    nc.scalar.copy(kt[:, cc, i * P:(i + 1) * P], tp[:, :])
        for i in range(n_blk):
            for cc in range(n_cch):
                tp = tpsum.tile([P, P], F32, tag="tp")
                nc.tensor.transpose(tp[:, :], qnat[:, i, cc * P:(cc + 1) * P],
                                    ident[:, :])
                if (i * n_cch + cc) % 2 == 0:
                    nc.vector.tensor_copy(qt[:, cc, i * P:(i + 1) * P], tp[:, :])
                else:
                    nc.scalar.copy(qt[:, cc, i * P:(i + 1) * P], tp[:, :])

    nc.vector.memset(vt[:, :, c:c + 1], 1.0)
    for i in range(n_blk):
        nc.vector.tensor_copy(vt[:, i, 0:c], qnat[:, i, :])  # fp32 -> bf16

    # --- stage B: attention, one query superblock at a time ---
    with (
        tc.tile_pool(name="etpsum", bufs=3, space="PSUM") as etp,
        tc.tile_pool(name="accpsum", bufs=4, space="PSUM") as accp,
    ):
        for sb in range(n_sb):
            nsl = slice(sb * SB, (sb + 1) * SB)
            acc = [accp.tile([P, c + 1], F32, tag="acc", name=f"acc{i}")
                   for i in range(nb_per_sb)]
            ats = [None] * n_blk

            def emit_et(mb):
                et = etp.tile([P, SB], F32, tag="et")
                for cc in range(n_cch):
                    nc.tensor.matmul(
                        et[:, :],
                        lhsT=kt[:, cc, mb * P:(mb + 1) * P],
                        rhs=qt[:, cc, nsl],
                        start=(cc == 0),
                        stop=(cc == n_cch - 1),
                    )
                at = atp.tile([P, SB], BF16, tag="at")
                nc.scalar.activation(out=at[:, :], in_=et[:, :],
                                     func=mybir.ActivationFunctionType.Exp,
                                     bias=shift_t[:, :], scale=-1.0)
                ats[mb] = at

            def emit_acc(mb):
                at = ats[mb]
                for nb in range(nb_per_sb):
                    nc.tensor.matmul(
                        acc[nb][:, :],
                        lhsT=at[:, nb * P:(nb + 1) * P],
                        rhs=vt[:, mb, :],
                        start=(mb == 0),
                        stop=(mb == n_blk - 1),
                    )
                ats[mb] = None

            # software-pipelined emission: PE queue = et0, et1, acc0, et2, ...
            emit_et(0)
            for mb in range(n_blk):
                if mb + 1 < n_blk:
                    emit_et(mb + 1)
                emit_acc(mb)

            for nb in range(nb_per_sb):
                blk = sb * nb_per_sb + nb
                inv = small.tile([P, 1], F32, tag="inv")
                nc.vector.reciprocal(inv[:, :], acc[nb][:, c:c + 1])
                sc = small.tile([P, 1], F32, tag="sc")
                nc.vector.tensor_mul(sc[:, :], inv[:, :], gt[:, :])
                ot = opool.tile([P, c], F32, tag="ot")
                nc.vector.tensor_scalar(
                    out=ot[:, :], in0=acc[nb][:, 0:c],
                    scalar1=sc[:, :], scalar2=None,
                    op0=mybir.AluOpType.mult,
                )
                nc.vector.tensor_add(ot[:, :], ot[:, :], qnat[:, blk, :])
                nc.default_dma_engine.dma_start(
                    out=o[blk * P:(blk + 1) * P, :], in_=ot[:, :]
                )


def build_bass(n=N, c=C, score_dtype=F32R):
    nc = bacc.Bacc("TRN2", target_bir_lowering=False, debug=False)
    q = nc.dram_tensor("q", [n, c], F32, kind="ExternalInput")
    k = nc.dram_tensor("k", [n, c], F32, kind="ExternalInput")
    g = nc.dram_tensor("gamma", [1, 1], F32, kind="ExternalInput")
    o = nc.dram_tensor("o", [n, c], F32, kind="ExternalOutput")
    with tile.TileContext(nc) as tc, ExitStack() as ctx:
        emit_cross_attn(ctx, tc, q[:], k[:], g, o[:], n, c, score_dtype)
    nc.compile()
    return nc


_CACHED_NC = None


def _get_nc():
    global _CACHED_NC
    if _CACHED_NC is None:
        _CACHED_NC = build_bass()
    return _CACHED_NC


def make_in_maps(xa, xb, gamma):
    xa = np.ascontiguousarray(np.asarray(xa, dtype=np.float32))
    xb = np.ascontiguousarray(np.asarray(xb, dtype=np.float32))
    g = np.full((1, 1), np.float32(np.asarray(gamma)), dtype=np.float32)
    in_maps = []
    for src_q, src_k in ((xa, xb), (xb, xa)):
        for b in range(B):
            in_maps.append({
                "q": np.ascontiguousarray(src_q[b].reshape(N, C)),
                "k": np.ascontiguousarray(src_k[b].reshape(N, C)),
                "gamma": g,
            })
    return in_maps


def assemble_out(results):
    outs = [np.asarray(r["o"]).reshape(H, W, C) for r in results]
    out_a = np.stack(outs[:B]).astype(np.float32)
    out_b = np.stack(outs[B:]).astype(np.float32)
    return out_a, out_b


def kernel(xa, xb, gamma, **run_kwargs):
    nc = _get_nc()
    res = run_bass_kernel_spmd(nc, make_in_maps(xa, xb, gamma),
                               core_ids=list(range(8)), **run_kwargs)
    out = assemble_out(res.results)
    if run_kwargs:
        return out, res
    return out


# revision 19
# speedup vs baseline: 6.0453x; 6.0453x over previous
"""Trainium2 Bass kernel for nn_CrossAttn (two-branch full cross attention).

Problem (per branch, per batch):
    q = x_q.reshape(N, C); k = x_k.reshape(N, C)          # N=4096, C=256
    E = q @ k.T                                           # [N, N]
    A = softmax(-E, axis=-1)
    out = gamma * (A @ q) + q                             # values == q

Sharding: 8 independent work units = 2 branches x 4 batches -> one per
NeuronCore (pure SPMD, no collectives).

Per-core dataflow:
  - Load q, k naturally; build V' = [q(bf16) | ones-column] per 128-row chunk.
  - PE-transpose q, k into Q^T, K^T ([c on partitions, n free], fp32).
  - Scores are computed TRANSPOSED: E_T[m, n] = sum_c K[m,c] Q[n,c]
    (lhsT = K^T chunk, rhs = Q^T superblock, fp32r full-rate).
  - A_T = exp(-E_T - SHIFT) on ScalarE (softmax is shift-invariant, so a
    constant shift replaces the row-max pass; -E ~ N(0,256) keeps
    exp(-E-100) far away from both fp32 overflow and total underflow).
  - out' = A_T.T @ V' accumulated over key chunks in PSUM; the ones
    column makes out'[:, C] the softmax denominator for free.
  - out = gamma * out'[:, :C] / out'[:, C] + q  (VectorE epilogue).
"""

from contextlib import ExitStack

import numpy as np

import concourse.bass as bass
import concourse.bacc as bacc
import concourse.mybir as mybir
import concourse.tile as tile
from concourse.bass_utils import run_bass_kernel_spmd
from concourse.masks import make_identity

F32 = mybir.dt.float32
F32R = mybir.dt.float32r
BF16 = mybir.dt.bfloat16

B, H, W, C = 4, 64, 64, 256
N = H * W  # 4096
SHIFT = -100.0  # constant softmax shift: A = exp(-E + SHIFT)


def emit_cross_attn(ctx, tc, q, k, g, o, n, c, score_dtype=F32R):
    """Emit one core's cross-attention program.

    q, k: DRAM [n, c] fp32 (q is queries+values+residual, k is keys)
    g:    DRAM [1, 1] fp32 (gamma)
    o:    DRAM [n, c] fp32
    """
    nc = tc.nc
    P = 128
    n_blk = n // P          # 128-row chunks of q/k
    n_cch = c // P          # 128-col chunks of the feature dim
    SB = min(512, n)        # query superblock width
    n_sb = n // SB
    nb_per_sb = SB // P
    # First chunk small so PE transposes start ASAP after kernel entry.
    if n_blk % 4 == 0 and n_blk >= 8:
        q4 = n_blk // 4
        chunk_sizes = [2, q4 - 2, q4, q4, q4]
    else:
        chunk_sizes = [n_blk]

    persist = ctx.enter_context(tc.tile_pool(name="persist", bufs=1))
    small = ctx.enter_context(tc.tile_pool(name="small", bufs=8))
    atp = ctx.enter_context(tc.tile_pool(name="atp", bufs=3))
    opool = ctx.enter_context(tc.tile_pool(name="opool", bufs=4))

    # --- persistent SBUF tensors ---
    ident = persist.tile([P, P], F32, tag="ident")
    make_identity(nc, ident[:, :])
    shift_t = persist.tile([P, 1], F32, tag="shift")
    nc.vector.memset(shift_t[:, :], SHIFT)
    gt = persist.tile([P, 1], F32, tag="gamma")
    g_ap = g[:]
    nc.default_dma_engine.dma_start(
        out=gt[:, :],
        in_=bass.AP(tensor=g_ap.tensor, offset=0, ap=[[0, P], [1, 1]]),
    )

    qnat = persist.tile([P, n_blk, c], F32, tag="qnat")   # q natural [p, blk, c]
    knat = persist.tile([P, n_blk, c], F32, tag="knat")
    qt = persist.tile([P, n_cch, n], score_dtype, tag="qt")  # Q^T [c, cch, n]
    kt = persist.tile([P, n_cch, n], score_dtype, tag="kt")
    vt = persist.tile([P, n_blk, c + 1], BF16, tag="vt")  # V' [m-part, blk, c+1]

    # --- stage A: load + transpose + build V' ---
    q3 = q.rearrange("(i p) c -> p i c", p=P)
    k3 = k.rearrange("(i p) c -> p i c", p=P)
    pos = 0
    for sz in chunk_sizes:
        sl = slice(pos, pos + sz)
        nc.default_dma_engine.dma_start(out=knat[:, sl, :], in_=k3[:, sl, :])
        pos += sz
    pos = 0
    for sz in chunk_sizes:
        sl = slice(pos, pos + sz)
        nc.default_dma_engine.dma_start(out=qnat[:, sl, :], in_=q3[:, sl, :])
        pos += sz

    # Transpose in groups of `tg` blocks: tg PE transposes land side by side
    # in one PSUM bank, then one wide PSUM->SBUF copy (amortizes per-op cost).
    tg = min(4, n_blk)
    with tc.tile_pool(name="tpsum", bufs=4, space="PSUM") as tpsum:
        grp = 0
        for src, dst in ((knat, kt), (qnat, qt)):  # K first: gates stage B
            for cc in range(n_cch):
                for i0 in range(0, n_blk, tg):
                    tp = tpsum.tile([P, tg * P], F32, tag="tp")
                    for j in range(tg):
                        nc.tensor.transpose(
                            tp[:, j * P:(j + 1) * P],
                            src[:, i0 + j, cc * P:(cc + 1) * P],
                            ident[:, :])
                    dst_sl = dst[:, cc, i0 * P:(i0 + tg) * P]
                    if grp % 2 == 0:
                        nc.vector.tensor_copy(dst_sl, tp[:, :])
                    else:
                        nc.scalar.copy(dst_sl, tp[:, :])
                    grp += 1

    nc.vector.memset(vt[:, :, c:c + 1], 1.0)
    for i in range(n_blk):
        nc.vector.tensor_copy(vt[:, i, 0:c], qnat[:, i, :])  # fp32 -> bf16

    # --- stage B: attention, one query superblock at a time ---
    with (
        tc.tile_pool(name="etpsum", bufs=4, space="PSUM") as etp,
        tc.tile_pool(name="accpsum", bufs=4, space="PSUM") as accp,
    ):
        for sb in range(n_sb):
            nsl = slice(sb * SB, (sb + 1) * SB)
            acc = [accp.tile([P, c + 1], F32, tag="acc", name=f"acc{i}")
                   for i in range(nb_per_sb)]
            ats = [None] * n_blk

            def emit_et(mb):
                et = etp.tile([P, SB], F32, tag="et")
                for cc in range(n_cch):
                    nc.tensor.matmul(
                        et[:, :],
                        lhsT=kt[:, cc, mb * P:(mb + 1) * P],
                        rhs=qt[:, cc, nsl],
                        start=(cc == 0),
                        stop=(cc == n_cch - 1),
                    )
                at = atp.tile([P, SB], BF16, tag="at")
                nc.scalar.activation(out=at[:, :], in_=et[:, :],
                                     func=mybir.ActivationFunctionType.Exp,
                                     bias=shift_t[:, :], scale=-1.0)
                ats[mb] = at

            def emit_acc(mb):
                at = ats[mb]
                for nb in range(nb_per_sb):
                    nc.tensor.matmul(
                        acc[nb][:, :],
                        lhsT=at[:, nb * P:(nb + 1) * P],
                        rhs=vt[:, mb, :],
                        start=(mb == 0),
                        stop=(mb == n_blk - 1),
                    )
                ats[mb] = None

            # software-pipelined emission, 2-deep lookahead:
            # PE queue = et0, et1, et2, acc0, et3, acc1, ...
            emit_et(0)
            if n_blk > 1:
                emit_et(1)
            for mb in range(n_blk):
                if mb + 2 < n_blk:
                    emit_et(mb + 2)
                emit_acc(mb)

            for nb in range(nb_per_sb):
                blk = sb * nb_per_sb + nb
                inv = small.tile([P, 1], F32, tag="inv")
                nc.vector.reciprocal(inv[:, :], acc[nb][:, c:c + 1])
                sc = small.tile([P, 1], F32, tag="sc")
                nc.vector.tensor_mul(sc[:, :], inv[:, :], gt[:, :])
                ot = opool.tile([P, c], F32, tag="ot")
                nc.vector.tensor_scalar(
                    out=ot[:, :], in0=acc[nb][:, 0:c],
                    scalar1=sc[:, :], scalar2=None,
                    op0=mybir.AluOpType.mult,
                )
                nc.vector.tensor_add(ot[:, :], ot[:, :], qnat[:, blk, :])
                nc.default_dma_engine.dma_start(
                    out=o[blk * P:(blk + 1) * P, :], in_=ot[:, :]
                )


def build_bass(n=N, c=C, score_dtype=F32R):
    nc = bacc.Bacc("TRN2", target_bir_lowering=False, debug=False)
    q = nc.dram_tensor("q", [n, c], F32, kind="ExternalInput")
    k = nc.dram_tensor("k", [n, c], F32, kind="ExternalInput")
    g = nc.dram_tensor("gamma", [1, 1], F32, kind="ExternalInput")
    o = nc.dram_tensor("o", [n, c], F32, kind="ExternalOutput")
    with tile.TileContext(nc) as tc, ExitStack() as ctx:
        emit_cross_attn(ctx, tc, q[:], k[:], g, o[:], n, c, score_dtype)
    nc.compile()
    return nc


_CACHED_NC = None


def _get_nc():
    global _CACHED_NC
    if _CACHED_NC is None:
        _CACHED_NC = build_bass()
    return _CACHED_NC


def make_in_maps(xa, xb, gamma):
    xa = np.ascontiguousarray(np.asarray(xa, dtype=np.float32))
    xb = np.ascontiguousarray(np.asarray(xb, dtype=np.float32))
    g = np.full((1, 1), np.float32(np.asarray(gamma)), dtype=np.float32)
    in_maps = []
    for src_q, src_k in ((xa, xb), (xb, xa)):
        for b in range(B):
            in_maps.append({
                "q": np.ascontiguousarray(src_q[b].reshape(N, C)),
                "k": np.ascontiguousarray(src_k[b].reshape(N, C)),
                "gamma": g,
            })
    return in_maps


def assemble_out(results):
    outs = [np.asarray(r["o"]).reshape(H, W, C) for r in results]
    out_a = np.stack(outs[:B]).astype(np.float32)
    out_b = np.stack(outs[B:]).astype(np.float32)
    return out_a, out_b


def kernel(xa, xb, gamma, **run_kwargs):
    nc = _get_nc()
    res = run_bass_kernel_spmd(nc, make_in_maps(xa, xb, gamma),
                               core_ids=list(range(8)), **run_kwargs)
    out = assemble_out(res.results)
    if run_kwargs:
        return out, res
    return out
